# revision 21
# baseline (speedup 1.0000x reference)
"""Converse2D-Up (FFT deconvolution upsampler) as a Bass/Tile kernel for TRN2.

Math (same factorization as the validated baseline, new engine mapping):
out_dd = real(IFFT132(Kdd_hat . Y)) per polyphase dd, Y = G x G^T with
G = F132 @ P (132x128).  Hermitian symmetry keeps only v=0..66 spectral
columns.  All matmuls run as float32r (TF32-class, 1 cyc/col at K=128,
N>=256), with constants or per-image tiles zero-padded so every
contraction is K=128:

  A : r1[m,u]    = x^T [Gr^T|Gi^T]                 (1 mm, N=264)
  B : Y^T[v,u]   = Gr67^T r1 + Gi67^T r1s          (2 mm, N=264)
  cmul (DVE)     : fx^T[v,(p,u)] = Kdd_hat . Y     (4 ops)
  D1': S[u,(y|y)] per phase = fx^T-slice^T [WC|WS] (8 mm, N=256)
  lo : u=128..131 handled batched per channel: 4 small mm -> S^T-lo,
       16 gathered PE transposes -> [8,(p,y)], K-padded to 128
  D2 : out[x,(p,y)] = CmT^T S_r - SmT^T S_i + lo   (3 mm, N=512)
  gelu+interleave on ScalarE eviction, row-contiguous DMA out.

Sharding: 8 channels per core x 4 batch images; weight/bias spectra are
host-precomputed constants.
"""

import os

import numpy as np

import concourse.bass as bass
import concourse.mybir as mybir
import concourse.tile as tile
from concourse import bacc
from concourse.bass import ts
from concourse.bass_utils import run_bass_kernel_spmd

F32 = mybir.dt.float32
F32R = mybir.dt.float32r
BF16 = mybir.dt.bfloat16
AF = mybir.ActivationFunctionType

SCALE = 2
PAD = 2
EPS = 1e-5
N0 = 128           # input spatial size
NP = N0 + 2 * PAD  # 132 padded
NU = NP * SCALE    # 264 upsampled
NV = NP // 2 + 1   # 67 unique spectral columns
B = 4
C = 64
NCORES = 8
CPC = C // NCORES  # 8 channels per core
NIMG = B * CPC     # 32 images per core

LAST_EXEC_NS = None  # set by kernel() when tracing is enabled


# --------------------------------------------------------------------------
# host-side constant precompute (weight/bias -> per-channel spectra)
# --------------------------------------------------------------------------

def _host_constants(weight, bias):
    w64 = np.asarray(weight, dtype=np.float64)
    b64 = np.asarray(bias, dtype=np.float64)

    # FB = p2o(weight): 264-point OTF of the rolled 3x3 PSF, per channel
    k_h, k_w = w64.shape[-2:]
    otf = np.zeros((C, NU, NU), dtype=np.complex128)
    otf[:, :k_h, :k_w] = w64[0]
    otf = np.roll(otf, (-(k_h // 2), -(k_w // 2)), axis=(-2, -1))
    FB = np.fft.fftn(otf, axes=(-2, -1))                      # (C,264,264)

    biaseps = 1.0 / (1.0 + np.exp(-(b64.reshape(C) - 9.0))) + EPS  # (C,)
    be = biaseps[:, None, None]

    u = np.arange(NU)
    Dr = 1 + np.exp(-2j * np.pi * u / NU)
    D = Dr[:, None] * Dr[None, :]                             # (264,264)

    Gh = np.conj(FB) + be * D[None]
    FBG = FB * Gh

    def quadmean(A):
        return 0.25 * (A[:, :NP, :NP] + A[:, NP:, :NP]
                       + A[:, :NP, NP:] + A[:, NP:, NP:])

    M1 = quadmean(FBG)
    invW = quadmean(np.abs(FB) ** 2)
    M2 = M1 / (invW + be)
    H = (Gh - np.conj(FB) * np.tile(M2, (1, SCALE, SCALE))) / be   # (C,264,264)

    hr = np.fft.ifft2(H, axes=(-2, -1)).real                  # H Hermitian
    # polyphase spectra: kdd[c,dx,dy,u,v] = FFT132(hr[c, dx::2, dy::2])[:, :NV]
    kdd = np.empty((C, 2, 2, NP, NV), dtype=np.complex128)
    for dx in range(2):
        for dy in range(2):
            kh = np.fft.fft2(hr[:, dx::2, dy::2], axes=(-2, -1))
            kdd[:, dx, dy] = kh[:, :, :NV]

    # kt packing: [c, v, (plane, p, u)] planes kt1=(kr,ki), kt2=(ki,kr)
    krT = np.ascontiguousarray(kdd.real.transpose(0, 4, 1, 2, 3)
                               ).reshape(C, NV, 4 * NP)       # (C,67,528)
    kiT = np.ascontiguousarray(kdd.imag.transpose(0, 4, 1, 2, 3)
                               ).reshape(C, NV, 4 * NP)
    bf16 = mybir.dt.np(mybir.dt.bfloat16)
    kt1 = np.concatenate([krT, kiT], axis=2).astype(bf16)  # (C,67,1056)
    kt2 = np.concatenate([kiT, krT], axis=2).astype(bf16)

    # forward matrix G = F132 @ P  (132x128 complex)
    P = np.zeros((NP, N0))
    for m in range(NP):
        P[m, (m - PAD) % N0] = 1.0
    F132 = np.exp(-2j * np.pi * np.outer(np.arange(NP), np.arange(NP)) / NP)
    G = F132 @ P

    gtr = np.concatenate([G.real.T, G.imag.T], axis=1).astype(np.float32)  # (128,264)
    g67 = np.concatenate([G.real.T[:, :NV], G.imag.T[:, :NV]],
                         axis=1).astype(np.float32)            # (128,134)

    # inverse matrix, rows x in [2,130) of iF132/132
    Ai = np.exp(2j * np.pi * np.outer(np.arange(2, 130), np.arange(NP)) / NP) / NP
    Cm, Sm = Ai.real, Ai.imag                                  # (128,132)

    w_v = np.ones(NV)
    w_v[1:NV - 1] = 2.0
    WC = (Cm[:, :NV] * w_v[None, :]).T                         # (67,128)
    WS = (Sm[:, :NV] * w_v[None, :]).T

    def pad128(a):
        out = np.zeros((128, a.shape[1]), dtype=np.float32)
        out[:a.shape[0]] = a
        return out

    wcws = pad128(np.concatenate([WC, WS], axis=1)).astype(bf16)   # (128,256)
    nwswc = pad128(np.concatenate([-WS, WC], axis=1)).astype(bf16)

    cmt = np.concatenate([Cm[:, :128].T, -Sm[:, :128].T],
                         axis=1).astype(bf16)                  # (128,256)
    cmlo = np.zeros((128, 128), dtype=np.float32)  # cast to bf16 below
    for j in range(4):
        cmlo[j] = Cm[:, 128 + j]          # row (ri=0, j)
        cmlo[4 + j] = -Sm[:, 128 + j]     # row (ri=1, j)

    return {
        "kt1": kt1, "kt2": kt2,
        "gtr": gtr, "g67": g67,
        "wcws": wcws, "nwswc": nwswc,
        "cmt": cmt, "cmlo": cmlo.astype(bf16),
        "ident": np.eye(128, dtype=np.float32),
    }


# --------------------------------------------------------------------------
# device kernel
# --------------------------------------------------------------------------

def build_nc(n_chan=CPC, n_batch=B, gelu=True):
    act_fn = AF.Gelu if gelu else AF.Copy
    n_img = n_chan * n_batch
    nc = bacc.Bacc("TRN2", target_bir_lowering=False, debug=False,
                   enable_asserts=False)

    x_t = nc.dram_tensor("x", [n_img, N0, N0], F32, kind="ExternalInput")
    kt1_t = nc.dram_tensor("kt1", [n_chan, NV, 8 * NP], BF16,
                           kind="ExternalInput")
    kt2_t = nc.dram_tensor("kt2", [n_chan, NV, 8 * NP], BF16,
                           kind="ExternalInput")
    gtr_t = nc.dram_tensor("gtr", [128, 2 * NP], F32, kind="ExternalInput")
    g67_t = nc.dram_tensor("g67", [128, 2 * NV], F32, kind="ExternalInput")
    wcws_t = nc.dram_tensor("wcws", [128, 256], BF16, kind="ExternalInput")
    nwswc_t = nc.dram_tensor("nwswc", [128, 256], BF16, kind="ExternalInput")
    cmt_t = nc.dram_tensor("cmt", [128, 256], BF16, kind="ExternalInput")
    cmlo_t = nc.dram_tensor("cmlo", [128, 128], BF16, kind="ExternalInput")
    id_t = nc.dram_tensor("ident", [128, 128], F32, kind="ExternalInput")
    out_t = nc.dram_tensor("out", [n_img, 2 * N0, 2 * N0], F32,
                           kind="ExternalOutput")

    P4 = 4 * NP   # 528
    from contextlib import ExitStack
    with tile.TileContext(nc) as tc:
        with ExitStack() as stack:
            pool = lambda name, bufs, **kw: stack.enter_context(
                tc.tile_pool(name=name, bufs=bufs, **kw))
            cstage = pool("cstage", 1)
            cpool = pool("consts", 1)
            ktpool = pool("kt", 2)
            xpool = pool("xin", 3)
            xrpool = pool("xr", 2)
            r1pool = pool("r1", 2)
            prodpool = pool("prod", 2)
            fxpool = pool("fx", 2)
            ssbpool = pool("ssb", 2)
            slopool = pool("slo", 2)
            lokpool = pool("lok", 1)
            opool = pool("osb", 3)
            ppa_pool = pool("ppa", 1, space="PSUM")
            ppb_pool = pool("ppb", 1, space="PSUM")
            pd1_pool = pool("pd1", 1, space="PSUM")
            plo_pool = pool("plo", 1, space="PSUM")
            plt_pool = pool("plt", 2, space="PSUM")
            ppd_pool = pool("ppd", 1, space="PSUM")
            # ---- constants: DMA fp32 staging, round once into F32R ----
            def cround(t_dram, shape, tag):
                stg = cstage.tile(shape, F32, tag="stg_" + tag)
                nc.sync.dma_start(stg[:], t_dram[:])
                dst = cpool.tile(shape, F32R, tag=tag)
                nc.scalar.activation(dst[:], stg[:], AF.Copy)
                return dst

            gtr = cround(gtr_t, [128, 2 * NP], "gtr")
            g67 = cround(g67_t, [128, 2 * NV], "g67")
            wcws = cpool.tile([128, 256], BF16, tag="wcws")
            nc.sync.dma_start(wcws[:], wcws_t[:])
            nwswc = cpool.tile([128, 256], BF16, tag="nwswc")
            nc.sync.dma_start(nwswc[:], nwswc_t[:])
            cmt = cpool.tile([128, 256], BF16, tag="cmt")
            nc.sync.dma_start(cmt[:], cmt_t[:])
            cmlo = cpool.tile([128, 128], BF16, tag="cmlo")
            nc.sync.dma_start(cmlo[:], cmlo_t[:])
            ident = cpool.tile([128, 128], F32)
            nc.sync.dma_start(ident[:], id_t[:])

            first_fx = [True, True]   # zero pad rows once per fx buffer
            first_lok = [True] * 4
            first_lg = [True, True]

            def emit_ab(ci):
                kt1 = ktpool.tile([NV, 8 * NP], BF16, tag="kt1")
                nc.sync.dma_start(kt1[:], kt1_t[ci])
                kt2 = ktpool.tile([NV, 8 * NP], BF16, tag="kt2")
                nc.sync.dma_start(kt2[:], kt2_t[ci])

                fxr4 = fxpool.tile([128, n_batch * P4], BF16, tag="fxr4")
                fxi4 = fxpool.tile([128, n_batch * P4], BF16, tag="fxi4")
                if first_fx[ci % 2]:
                    # rows 67:128 are K-padding read by matmuls; must not be
                    # NaN (partition base must be 32-aligned, so clear 64:128)
                    nc.vector.memset(fxr4[64:128, :], 0.0)
                    nc.vector.memset(fxi4[64:128, :], 0.0)
                    first_fx[ci % 2] = False

                # ---- stage A+B per image ----
                for bi in range(n_batch):
                    img = ci * n_batch + bi
                    xf = xpool.tile([N0, N0], F32, tag="x")
                    nc.sync.dma_start(xf[:], x_t[img])
                    xr = xrpool.tile([N0, N0], F32R, tag="xr")
                    nc.gpsimd.tensor_copy(xr[:], xf[:])

                    pA = ppa_pool.tile([128, 2 * NP], F32, tag="pA")
                    nc.tensor.matmul(pA[:], xr[:], gtr[:], start=True,
                                     stop=True)

                    r1 = r1pool.tile([128, 2 * NP], F32R, tag="r1")
                    nc.scalar.activation(r1[:], pA[:], AF.Copy)
                    r1s = r1pool.tile([128, 2 * NP], F32R, tag="r1s")
                    nc.scalar.activation(r1s[:, 0:NP], pA[:, NP:2 * NP],
                                         AF.Copy, scale=-1.0)
                    nc.vector.tensor_copy(r1s[:, NP:2 * NP], pA[:, 0:NP])

                    pB = ppb_pool.tile([NV, 2 * NP], F32, tag="pB")
                    nc.tensor.matmul(pB[:], g67[:, 0:NV], r1[:],
                                     start=True, stop=False)
                    nc.tensor.matmul(pB[:], g67[:, NV:2 * NV], r1s[:],
                                     start=False, stop=True)

                    # ---- cmul: fx^T[v,(p,u)]; Y staged to SBUF so the
                    # imag half can run on GpSimd (which cannot read PSUM)
                    yb = r1pool.tile([NV, 2 * NP], BF16, tag="yb")
                    nc.scalar.activation(yb[:], pB[:], AF.Copy)
                    ybc = (yb[:]
                           .rearrange("v (pl u) -> v pl u", pl=2)
                           [:, :, None, :]
                           .broadcast_to([NV, 2, 4, NP]))
                    prodA = prodpool.tile([NV, 8 * NP], BF16, tag="prodA")
                    nc.vector.tensor_mul(
                        prodA[:].rearrange("v (pl p u) -> v pl p u",
                                           pl=2, p=4),
                        kt1[:].rearrange("v (pl p u) -> v pl p u",
                                         pl=2, p=4),
                        ybc)
                    nc.vector.tensor_sub(fxr4[0:NV, ts(bi, P4)],
                                         prodA[:, 0:P4], prodA[:, P4:2 * P4])
                    prodB = prodpool.tile([NV, 8 * NP], BF16, tag="prodB")
                    nc.gpsimd.tensor_mul(
                        prodB[:].rearrange("v (pl p u) -> v pl p u",
                                           pl=2, p=4),
                        kt2[:].rearrange("v (pl p u) -> v pl p u",
                                         pl=2, p=4),
                        ybc)
                    nc.gpsimd.tensor_add(fxi4[0:NV, ts(bi, P4)],
                                         prodB[:, 0:P4], prodB[:, P4:2 * P4])
                return fxr4, fxi4

            def emit_rest(ci, fxr4, fxi4):
                pbs = []
                # ---- D1' + Ssb eviction per image ----
                for bi in range(n_batch):
                    b0 = bi * P4
                    ps01 = pd1_pool.tile([128, 512], F32, tag="ps01")
                    ps23 = pd1_pool.tile([128, 512], F32, tag="ps23")
                    for p in range(4):
                        ps = ps01 if p < 2 else ps23
                        o = ps[:, ts(p % 2, 256)]
                        u0 = b0 + p * NP
                        nc.tensor.matmul(o, fxr4[:, u0:u0 + 128], wcws[:],
                                         start=True, stop=False)
                        nc.tensor.matmul(o, fxi4[:, u0:u0 + 128], nwswc[:],
                                         start=False, stop=True)
                    # ssb layout [u, (ri, p, y)] so D2's moving APs are flat
                    ssb = ssbpool.tile([128, 1024], BF16, tag="ssb%d" % bi)
                    sv = ssb[:].rearrange("u (ri p y) -> u p ri y",
                                          ri=2, p=4)
                    for p in range(4):
                        ps = ps01 if p < 2 else ps23
                        src = (ps[:, ts(p % 2, 256)]
                               .rearrange("u (ri y) -> u ri y", ri=2))
                        if p % 2 == 0:
                            nc.scalar.activation(sv[:, p], src, AF.Copy)
                        else:
                            nc.vector.tensor_copy(sv[:, p], src)
                    pbs.append(ssb)

                # ---- lo rows u=128..131, batched over the 4 images ----
                # stage the strided (b,p,ulo) gather into a contiguous tile
                # (matmul operand APs must be single-free-dim)
                lg = slopool.tile([128, 128], BF16, tag="logath")
                if first_lg[ci % 2]:
                    nc.vector.memset(lg[:, :], 0.0)
                    first_lg[ci % 2] = False
                nc.vector.tensor_copy(
                    lg[0:NV, 0:64].rearrange("k (b p u) -> k b p u",
                                             b=n_batch, p=4),
                    fxr4[0:NV].rearrange("k (b p u) -> k b p u",
                                         b=n_batch, p=4)[:, :, :, 128:132])
                nc.vector.tensor_copy(
                    lg[0:NV, 64:128].rearrange("k (b p u) -> k b p u",
                                               b=n_batch, p=4),
                    fxi4[0:NV].rearrange("k (b p u) -> k b p u",
                                         b=n_batch, p=4)[:, :, :, 128:132])

                plo = plo_pool.tile([128, 128], F32, tag="plo")
                nc.tensor.matmul(plo[:, 0:64], wcws[:, 0:128], lg[:, 0:64],
                                 start=True, stop=False)
                nc.tensor.matmul(plo[:, 0:64], nwswc[:, 0:128], lg[:, 64:128],
                                 start=False, stop=True)
                nc.tensor.matmul(plo[:, 64:128], wcws[:, 128:256],
                                 lg[:, 0:64], start=True, stop=False)
                nc.tensor.matmul(plo[:, 64:128], wcws[:, 0:128],
                                 lg[:, 64:128], start=False, stop=True)
                # evict interleaved to [y, (b, p, ri, u)] so each transpose
                # source is a contiguous 8-column slice
                slo = slopool.tile([128, 128], F32, tag="slo")
                nc.scalar.activation(
                    slo[:].rearrange("y (b p ri u) -> y ri b p u",
                                     ri=2, b=n_batch, p=4),
                    plo[:].rearrange("y (ri b p u) -> y ri b p u",
                                     ri=2, b=n_batch, p=4),
                    AF.Copy)

                loks = []
                for bi in range(n_batch):
                    plt = plt_pool.tile([8, 512], F32, tag="plt")
                    for p in range(4):
                        src = slo[:, bi * 32 + p * 8:bi * 32 + p * 8 + 8]
                        nc.tensor.transpose(plt[:, ts(p, 128)], src,
                                            ident[:])
                    lok = lokpool.tile([128, 512], BF16, tag="lok%d" % bi)
                    if first_lok[bi]:
                        nc.vector.memset(lok[:, :], 0.0)
                        first_lok[bi] = False
                    nc.scalar.activation(lok[0:8, :], plt[:], AF.Copy)
                    loks.append(lok)

                # ---- D2 + gelu + store per image ----
                for bi in range(n_batch):
                    img = ci * n_batch + bi
                    ssb = pbs[bi]
                    pD = ppd_pool.tile([128, 512], F32, tag="pD")
                    nc.tensor.matmul(pD[:], cmt[:, 0:128], ssb[:, 0:512],
                                     start=True, stop=False)
                    nc.tensor.matmul(pD[:], cmt[:, 128:256],
                                     ssb[:, 512:1024],
                                     start=False, stop=False)
                    nc.tensor.matmul(pD[:], cmlo[:], loks[bi][:],
                                     start=False, stop=True)

                    osb = opool.tile([128, 512], F32, tag="osb")
                    nc.scalar.activation(
                        osb[:].rearrange("x (dx y dy) -> x dx dy y",
                                         dx=2, dy=2),
                        pD[:].rearrange("x (dx dy y) -> x dx dy y",
                                        dx=2, dy=2),
                        act_fn)
                    dst = out_t[img].rearrange("(x dx) Y -> x dx Y", dx=2)
                    nc.sync.dma_start(dst, osb[:].rearrange(
                        "x (dx Y) -> x dx Y", dx=2))

            # software pipeline: queue channel c+1's A/B/cmul work on the
            # engines before channel c's D1'/lo/D2 so the PE never drains
            # while a channel's cmuls finish
            chst = {}
            for ci in range(n_chan):
                chst[ci] = emit_ab(ci)
                if ci > 0:
                    emit_rest(ci - 1, *chst.pop(ci - 1))
            emit_rest(n_chan - 1, *chst.pop(n_chan - 1))

    nc.compile()
    return nc


# --------------------------------------------------------------------------
# public entry point: full inputs in, full output out
# --------------------------------------------------------------------------

def kernel(x, weight, bias):
    global LAST_EXEC_NS
    x = np.ascontiguousarray(np.asarray(x, dtype=np.float32))
    consts = _host_constants(weight, bias)

    nc = build_nc()

    in_maps = []
    for core in range(NCORES):
        c0 = core * CPC
        xs = np.ascontiguousarray(
            x[:, c0:c0 + CPC].transpose(1, 0, 2, 3)).reshape(NIMG, N0, N0)
        in_maps.append({
            "x": xs,
            "kt1": np.ascontiguousarray(consts["kt1"][c0:c0 + CPC]),
            "kt2": np.ascontiguousarray(consts["kt2"][c0:c0 + CPC]),
            "gtr": consts["gtr"],
            "g67": consts["g67"],
            "wcws": consts["wcws"],
            "nwswc": consts["nwswc"],
            "cmt": consts["cmt"],
            "cmlo": consts["cmlo"],
            "ident": consts["ident"],
        })

    trace = os.environ.get("KERNEL_TRACE", "0") == "1"
    tmpdir = os.environ.get("KERNEL_TMPDIR") or None
    res = run_bass_kernel_spmd(nc, in_maps, list(range(NCORES)), trace=trace,
                               tmpdir=tmpdir)
    LAST_EXEC_NS = res.exec_time_ns

    out = np.empty((B, C, 2 * N0, 2 * N0), dtype=np.float32)
    for core in range(NCORES):
        c0 = core * CPC
        o = res.results[core]["out"].reshape(CPC, B, 2 * N0, 2 * N0)
        out[:, c0:c0 + CPC] = o.transpose(1, 0, 2, 3)
    return out
